# revision 61
# baseline (speedup 1.0000x reference)
"""Trainium2 Bass kernel for nn_DCModuleOptimized (pooling, b=32 512x512).

Math (validated vs the jax reference, rel MSE ~1.6e-3 < 2e-2):
  For comparison image c in {positive, negative}:
    - 9 shifted stride-2 downsampled planes k=(ky,kx) of e=a-c (255x255)
    - per group of 9 consecutive elements of the k-major flattened planes
      (groups never span planes: 65025 = 9*7225), select c at argmin and
      argmax of |e|;  s = c_argmin + c_argmax  (65025 values, l-ordered)
    - out[y,x] = s[min(y//2,254)*255 + min(x//2,254)] for y,x < 511, else 0.

Key-packing trick: keys = uint32 (bits(fp16 |e|) << 16) | bits(fp16 c).
For d >= 0 the fp16 bit pattern is monotone, so uint32 min/max over a group
selects by d and carries the fp16 c payload in the low half for free -- no
equality masks / index selection needed.  DVE computes u32 min/max through an
fp32 pipeline which rounds the low ~6 bits of c; that costs ~1e-3 rel MSE.

Engine split (min/max only exist on DVE):
  DVE:  min/max trees over keys (4+4 tensor_tensor ops per wave-ci)
  ACT:  |e| extraction into key high halves, c cast into low halves
  Pool: raw-space subtract a-c, final combine add, column-duplicate
  DMA:  fp16 inputs (host pre-cast), l-order scatter of duplicated rows to a
        DRAM scratch image, then DRAM->DRAM row-doubling assembly; fp16
        outputs (host casts back to fp32).

Sharding: pure data parallel, batch dim 32 -> 8 cores x 4.
"""
import numpy as np

import concourse.bass as bass
import concourse.mybir as mybir
import concourse.tile as tile
from concourse.vector_clock import ScopedClock

F16 = mybir.dt.float16
F32 = mybir.dt.float32
U32 = mybir.dt.uint32
AF = mybir.ActivationFunctionType
ALU = mybir.AluOpType

P = 85          # t-units per image (255 plane rows / 3)
RAW = 3584      # 7 raw rows of 512 per partition
CMP = 6885      # 9 planes x 765 compacted elements per partition
GRP = 765       # groups per partition (85 per plane x 9 planes)
WP = 128
IMG = 512 * 512


def _patched_drain_and_barrier(self, tick_clock, wait_clock):
    # This container's walrus rejects >1 sync-wait command per instruction;
    # emit the Tile tail waits as standalone single-wait instructions.
    nc = self.nc
    carrier = nc.sync.engine_nop() if hasattr(nc.sync, 'engine_nop') else nc.sync.nop()
    wait_clock.add_sem_waits(carrier.ins, ScopedClock({None: tick_clock.global_clock}))
    si = carrier.ins.sync_info
    waits = list(si.on_wait) if si else []
    carrier.ins.sync_info = mybir.SyncInfo(on_wait=[], on_update=[])
    sem_by_name = {h.name: h for h in self.sems.allocated().values()}
    for w in waits:
        nc.sync.wait_ge(sem_by_name[w.ant_name], w.wait_value)
    nc.sync.drain()
    nc.all_engine_barrier()
    popped = nc._tile_sem_poison_stack.pop()
    assert popped is self._sem_poison
    nc.clear_and_free_semaphores(list(self.sems.allocated().values()))
    nc.all_engine_barrier()


_MAXW = 1
_orig_add_instruction = tile.TileContext._add_instruction


def _split_add_instruction(self, inst):
    si = inst.sync_info
    if si is not None and len(si.on_wait) > _MAXW:
        waits = list(si.on_wait)
        head, tail = waits[:-_MAXW], waits[-_MAXW:]
        for i in range(0, len(head), _MAXW):
            chunk = head[i:i + _MAXW]
            wi = mybir.InstEventSemaphore(name=f"I-{self.nc.next_id()}", ins=[], outs=[])
            wi.engine = inst.engine
            wi.sync_info = mybir.SyncInfo(on_wait=chunk, on_update=[])
            _orig_add_instruction(self, wi)
        inst.sync_info = mybir.SyncInfo(on_wait=tail, on_update=list(si.on_update))
    _orig_add_instruction(self, inst)


def _install_patches():
    tile.TileContext._drain_and_barrier = _patched_drain_and_barrier
    tile.TileContext._add_instruction = _split_add_instruction


def _rap(t, offset, dims):
    return bass.AP(tensor=t.tensor if isinstance(t, bass.AP) else t, offset=offset, ap=dims)


def build(nb=4):
    _install_patches()
    nc = bass.Bass()
    nwaves = (nb * P + WP - 1) // WP
    anc = nc.declare_dram_parameter("anchor", [nwaves, WP, RAW], F16, isOutput=False)
    pos = nc.declare_dram_parameter("positive", [nwaves, WP, RAW], F16, isOutput=False)
    neg = nc.declare_dram_parameter("negative", [nwaves, WP, RAW], F16, isOutput=False)
    out_pos = nc.declare_dram_parameter("out_pos", [nb, 512, 512], F16, isOutput=True)
    out_neg = nc.declare_dram_parameter("out_neg", [nb, 512, 512], F16, isOutput=True)

    units = [(b, t) for b in range(nb) for t in range(P)]
    waves = [units[i:i + WP] for i in range(0, len(units), WP)]

    def segments(wave):
        segs = []
        i = 0
        while i < len(wave):
            b0 = wave[i][0]
            k = i
            while k < len(wave) and wave[k][0] == b0:
                k += 1
            segs.append((i, k, b0, wave[i][1]))
            i = k
        return segs

    with tile.TileContext(nc) as tc:
        with (
            tc.tile_pool(name="pa", bufs=2) as pa,
            tc.tile_pool(name="pc", bufs=2) as pc,
            tc.tile_pool(name="pe", bufs=2) as pe,
            tc.tile_pool(name="pk", bufs=1) as pk,
            tc.tile_pool(name="pk1", bufs=1) as pk1,
            tc.tile_pool(name="pt", bufs=1) as pt,
            tc.tile_pool(name="pm", bufs=2) as pm,
            tc.tile_pool(name="psd", bufs=2) as psd,
            tc.tile_pool(name="pz", bufs=1) as pz,
            tc.tile_pool(name="pdram", bufs=9, space="DRAM") as pdram,
        ):
            Z = pz.tile([1, 512], F16)
            nc.vector.memset(Z[:, :], 0.0)
            zrow = pdram.tile([1, 512], F16, name="zrow", tag="zrow")
            zb = zrow[:, :]

            def emit_zero_borders():
                nc.sync.dma_start(out=zrow[:, :], in_=Z[:, :])
                # constant zero borders (col 511 rows 0..510, row 511) for every
                # job -- emitted mid-run so their HWDGE cost stays off both the
                # startup loads and the drain tail
                for b in range(nb):
                    for dst in (out_pos, out_neg):
                        base = b * IMG
                        nc.sync.dma_start(
                            out=_rap(dst, base + 511, [[512, 511], [1, 1]]),
                            in_=_rap(zrow, zb.offset, [[0, 511], [1, 1]]))
                        nc.sync.dma_start(
                            out=_rap(dst, base + 511 * 512, [[1, 512]]),
                            in_=_rap(zrow, zb.offset, [[1, 512]]))

            scratch = {}

            def emit_loads(wi_, wave, chunked=False):
                # host ships inputs pre-arranged in (wave, partition) order:
                # one contiguous DMA per (wave, tensor).  chunked (first wave):
                # rows 0..5 land first so extraction can start earlier.
                A = pa.tile([WP, RAW], F16, name=f"a{id(wave)}")
                Cp = pc.tile([WP, RAW], F16, tag="cp", name=f"cp{id(wave)}")
                Cn = pc.tile([WP, RAW], F16, tag="cn", name=f"cn{id(wave)}")
                nw_ = len(wave)
                chunks = [(0, 3072), (3072, 512)] if chunked else [(0, RAW)]
                for (off, w_) in chunks:
                    for (tileT, srcT) in ((A, anc), (Cp, pos), (Cn, neg)):
                        tb = tileT[:, :]
                        nc.sync.dma_start(
                            out=_rap(tileT, tb.offset + off, [[tb.ap[0][0], nw_], [1, w_]]),
                            in_=_rap(srcT, wi_ * WP * RAW + off, [[RAW, nw_], [1, w_]]))
                return A, Cp, Cn

            def emit_esubs(wave, A, Cp, Cn, dve_ci0=False, chunked=False):
                nw_ = len(wave)
                es = []
                chunks = ([(0, [[1, 3072]]), (3072, [[1, 512]])] if chunked
                          else [(0, [[1, RAW]])])
                for ci, C in ((0, Cp), (1, Cn)):
                    eng = nc.vector if dve_ci0 else nc.gpsimd
                    E = pe.tile([WP, RAW], F16, tag=f"e{ci}", name=f"e{ci}_{id(wave)}")
                    for (off, dims) in chunks:
                        eng.tensor_tensor(
                            out=_rap(E, E[:, :].offset + off, [[E[:, :].ap[0][0], nw_]] + dims),
                            in0=_rap(A, A[:, :].offset + off, [[A[:, :].ap[0][0], nw_]] + dims),
                            in1=_rap(C, C[:, :].offset + off, [[C[:, :].ap[0][0], nw_]] + dims),
                            op=ALU.subtract)
                    es.append(E)
                return es

            loaded = emit_loads(0, waves[0], chunked=True)
            esubs = emit_esubs(waves[0], *loaded, dve_ci0=True, chunked=True)
            for wi, wave in enumerate(waves):
                nw = len(wave)
                segs = segments(wave)
                A, Cp, Cn = loaded
                cur_esubs = esubs
                if wi + 1 < len(waves):
                    loaded = emit_loads(wi + 1, waves[wi + 1])
                    esubs = emit_esubs(waves[wi + 1], *loaded)

                first_wave = wi == 0
                last_wave = wi == len(waves) - 1
                if wi == 1:
                    emit_zero_borders()
                for ci, C in ((0, Cp), (1, Cn)):
                    # fine-grained (per-ky) issue order at the pipeline fill and
                    # drain ends; coarse in steady state
                    split = first_wave or (last_wave and ci == 1)
                    E = cur_esubs[ci]

                    K = (pk if ci == 0 else pk1).tile([WP, CMP], U32, tag=f"k{ci}")
                    kb = K[0:nw, :]
                    Kf = K.tensor.bitcast(F16)
                    fpitch = Kf[:, :].ap[0][0]
                    eb = E[0:nw, :]
                    cb = C[0:nw, :]

                    def ext_abs(ky):
                        # key high halves: |e| per plane (ACT, strided extract)
                        nc.scalar.activation(
                            out=bass.AP(tensor=Kf, offset=2 * kb.offset + 4590 * ky + 1,
                                        ap=[[fpitch, nw], [1530, 3], [510, 3], [2, 255]]),
                            in_=_rap(E, eb.offset + 512 * ky,
                                     [eb.ap[0], [1, 3], [1024, 3], [2, 255]]),
                            func=AF.Abs)

                    def ext_c(ky, eng="act"):
                        # key low halves: fp16 c per plane
                        dst = bass.AP(tensor=Kf, offset=2 * kb.offset + 4590 * ky,
                                      ap=[[fpitch, nw], [1530, 3], [510, 3], [2, 255]])
                        srcap = _rap(C, cb.offset + 512 * ky,
                                     [cb.ap[0], [1, 3], [1024, 3], [2, 255]])
                        if eng == "pool":
                            nc.gpsimd.tensor_copy(dst, srcap)
                        elif eng == "dve":
                            nc.vector.tensor_copy(dst, srcap)
                        else:
                            nc.scalar.activation(dst, srcap, func=AF.Copy)


                    # min/max trees over uint32 keys (DVE only); g0/ng select a
                    # group range (whole wave or one ky block)
                    T4 = pt.tile([WP, 4 * GRP], U32, tag="t4")
                    t4b = T4[0:nw, :]
                    KMIN = pm.tile([WP, GRP], U32, tag=f"kmin{ci}")
                    KMAX = pm.tile([WP, GRP], U32, tag=f"kmax{ci}")
                    SD = psd.tile([WP, 2 * GRP], F16, tag=f"sd{ci}")
                    sdb = SD[0:nw, :]
                    KMINf = KMIN.tensor.bitcast(F16)
                    KMAXf = KMAX.tensor.bitcast(F16)
                    mpitch = KMINf[:, :].ap[0][0]
                    mb = KMIN[0:nw, :]
                    xb = KMAX[0:nw, :]

                    def kj(g0, ng, j, w):
                        return bass.AP(tensor=K.tensor, offset=kb.offset + 9 * g0 + j,
                                       ap=[kb.ap[0], [9, ng], [1, w]])

                    def t4(g0, ng, j, w):
                        return bass.AP(tensor=T4.tensor, offset=t4b.offset + 4 * g0 + j,
                                       ap=[t4b.ap[0], [4, ng], [1, w]])

                    def trees_head(g0, ng):
                        # L0 of both trees + full max tree (releases the K range)
                        v = nc.vector
                        v.tensor_tensor(out=t4(g0, ng, 0, 4), in0=kj(g0, ng, 0, 4), in1=kj(g0, ng, 4, 4), op=ALU.min)
                        v.tensor_tensor(out=kj(g0, ng, 4, 4), in0=kj(g0, ng, 4, 4), in1=kj(g0, ng, 0, 4), op=ALU.max)
                        v.tensor_tensor(out=kj(g0, ng, 6, 2), in0=kj(g0, ng, 6, 2), in1=kj(g0, ng, 4, 2), op=ALU.max)
                        v.tensor_tensor(out=kj(g0, ng, 7, 1), in0=kj(g0, ng, 7, 1), in1=kj(g0, ng, 6, 1), op=ALU.max)
                        v.tensor_tensor(out=_rap(KMAX, xb.offset + g0, [xb.ap[0], [1, ng]]),
                                        in0=kj(g0, ng, 7, 1), in1=kj(g0, ng, 8, 1), op=ALU.max)

                    def trees_min_tail(g0, ng):
                        # min tree levels 1..3 on T4 (independent of K except j=8)
                        v = nc.vector
                        v.tensor_tensor(out=t4(g0, ng, 0, 2), in0=t4(g0, ng, 0, 2), in1=t4(g0, ng, 2, 2), op=ALU.min)
                        v.tensor_tensor(out=t4(g0, ng, 0, 1), in0=t4(g0, ng, 0, 1), in1=t4(g0, ng, 1, 1), op=ALU.min)
                        v.tensor_tensor(out=_rap(KMIN, mb.offset + g0, [mb.ap[0], [1, ng]]),
                                        in0=t4(g0, ng, 0, 1), in1=kj(g0, ng, 8, 1), op=ALU.min)

                    def trees(g0, ng):
                        trees_head(g0, ng)
                        trees_min_tail(g0, ng)

                    comb_eng = nc.gpsimd

                    def combine(g0, ng):
                        # s = c_argmin + c_argmax into even slots of the
                        # column-duplicated row; odd slots = copy.
                        comb_eng.tensor_tensor(
                            out=_rap(SD, sdb.offset + 2 * g0, [sdb.ap[0], [2, ng]]),
                            in0=bass.AP(tensor=KMINf, offset=2 * (mb.offset + g0), ap=[[mpitch, nw], [2, ng]]),
                            in1=bass.AP(tensor=KMAXf, offset=2 * (xb.offset + g0), ap=[[mpitch, nw], [2, ng]]),
                            op=ALU.add)
                        comb_eng.tensor_copy(
                            _rap(SD, sdb.offset + 2 * g0 + 1, [sdb.ap[0], [2, ng]]),
                            _rap(SD, sdb.offset + 2 * g0, [sdb.ap[0], [2, ng]]))

                    if split:
                        for ky in range(3):
                            ext_c(ky)
                            ext_abs(ky)
                            trees(255 * ky, 255)
                            combine(255 * ky, 255)
                    else:
                        for ky in range(3):
                            ext_c(ky)
                            ext_abs(ky)
                            # L0 of both trees per ky (keeps extract->tree overlap)
                            nc.vector.tensor_tensor(out=t4(255 * ky, 255, 0, 4),
                                                    in0=kj(255 * ky, 255, 0, 4),
                                                    in1=kj(255 * ky, 255, 4, 4), op=ALU.min)
                            nc.vector.tensor_tensor(out=kj(255 * ky, 255, 4, 4),
                                                    in0=kj(255 * ky, 255, 4, 4),
                                                    in1=kj(255 * ky, 255, 0, 4), op=ALU.max)
                        # coarse tails for both trees
                        nc.vector.tensor_tensor(out=kj(0, GRP, 6, 2), in0=kj(0, GRP, 6, 2),
                                                in1=kj(0, GRP, 4, 2), op=ALU.max)
                        nc.vector.tensor_tensor(out=kj(0, GRP, 7, 1), in0=kj(0, GRP, 7, 1),
                                                in1=kj(0, GRP, 6, 1), op=ALU.max)
                        nc.vector.tensor_tensor(out=_rap(KMAX, xb.offset, [xb.ap[0], [1, GRP]]),
                                                in0=kj(0, GRP, 7, 1), in1=kj(0, GRP, 8, 1), op=ALU.max)
                        trees_min_tail(0, GRP)
                        combine(0, GRP)

                    # scatter duplicated rows into the job's DRAM scratch image
                    # (flat l-dup space: run (k,t) -> offset 2*(7225k+85t), len 170)
                    # drain wave: 3 per-ky scatters so they fire incrementally
                    kranges = [(0, 3), (3, 3), (6, 3)] if last_wave else [(0, 9)]
                    for (p0, p1, b, t0) in segs:
                        ji = (b, ci)
                        if ji not in scratch:
                            scratch[ji] = pdram.tile([255, 510], F16, name=f"fd{b}_{ci}", tag="fd")
                        fdb = scratch[ji][:, :]
                        npart = p1 - p0
                        for (k0, nk) in kranges:
                            nc.sync.dma_start(
                                out=_rap(scratch[ji], fdb.offset + 170 * t0 + 14450 * k0,
                                         [[170, npart], [14450, nk], [1, 170]]),
                                in_=bass.AP(tensor=SD.tensor, offset=SD[p0:p1, :].offset + 170 * k0,
                                            ap=[SD[p0:p1, :].ap[0], [170, nk], [1, 170]]))

                    # assembly for jobs whose last unit is in this wave
                    for (p0, p1, b, t0) in segs:
                        if t0 + (p1 - p0) < P:
                            continue
                        dst = out_pos if ci == 0 else out_neg
                        FD = scratch[(b, ci)]
                        fdb = FD[:, :]
                        base = b * IMG
                        # rows 0..509 x cols 0..509, each scratch row written twice
                        nc.sync.dma_start(
                            out=_rap(dst, base, [[1024, 255], [512, 2], [1, 510]]),
                            in_=_rap(FD, fdb.offset, [[510, 255], [0, 2], [1, 510]]))
                        # row 510 = scratch row 254
                        nc.sync.dma_start(
                            out=_rap(dst, base + 510 * 512, [[1, 510]]),
                            in_=_rap(FD, fdb.offset + 510 * 254, [[1, 510]]))
                        # corner (510, 510)
                        nc.sync.dma_start(
                            out=_rap(dst, base + 510 * 512 + 510, [[1, 1]]),
                            in_=_rap(FD, fdb.offset + 510 * 254 + 509, [[1, 1]]))
                        # col 510 for rows 0..509
                        nc.sync.dma_start(
                            out=_rap(dst, base + 510, [[512, 510], [1, 1]]),
                            in_=_rap(FD, fdb.offset + 509, [[510, 255], [0, 2], [1, 1]]))
                        del scratch[(b, ci)]
    return nc


_CACHED = {}


def kernel(anchor: np.ndarray, positive: np.ndarray, negative: np.ndarray):
    from concourse import bass_utils

    n_cores = 8
    b = anchor.shape[0]
    nb = b // n_cores
    key = (nb,)
    if key not in _CACHED:
        _CACHED[key] = build(nb)
    nc = _CACHED[key]

    anchor = anchor.astype(np.float16)
    positive = positive.astype(np.float16)
    negative = negative.astype(np.float16)

    # wave-partition input layout: unit (b, t) -> raw rows 6t..6t+7 of image b
    P_, WP_, RAW_ = 85, 128, 3584
    units = [(b, t) for b in range(nb) for t in range(P_)]
    nwaves = (len(units) + WP_ - 1) // WP_
    idx_b = np.zeros((nwaves, WP_), dtype=np.int64)
    start = np.zeros((nwaves, WP_), dtype=np.int64)
    for u, (b, t) in enumerate(units):
        idx_b[u // WP_, u % WP_] = b
        start[u // WP_, u % WP_] = 6 * 512 * t
    col = np.arange(RAW_)

    def wave_layout(x):
        flat = x.reshape(nb, -1)
        return np.ascontiguousarray(flat[idx_b[..., None], start[..., None] + col])

    in_maps = []
    for i in range(n_cores):
        sl = slice(i * nb, (i + 1) * nb)
        in_maps.append({"anchor": wave_layout(anchor[sl]),
                        "positive": wave_layout(positive[sl]),
                        "negative": wave_layout(negative[sl])})

    res = bass_utils.run_bass_kernel_spmd(nc, in_maps, list(range(n_cores)))
    out_pos = np.concatenate([res.results[i]["out_pos"] for i in range(n_cores)], axis=0).astype(np.float32)
    out_neg = np.concatenate([res.results[i]["out_neg"] for i in range(n_cores)], axis=0).astype(np.float32)
    return out_pos, out_neg
